# revision 32
# baseline (speedup 1.0000x reference)
"""DeepFM (embedding_lookup) Trainium2 Bass kernel.

Sharding: data-parallel on batch across 8 NeuronCores; the combined
embedding table (~177 MB) is replicated per core.

Self-contained: hardcodes all shapes from the problem spec.
"""

import numpy as np

import concourse.bass as bass
import concourse.bacc as bacc
import concourse.mybir as mybir
import concourse.tile as tile
from concourse.bass import IndirectOffsetOnAxis
from concourse.bass_utils import run_bass_kernel_spmd
from concourse.masks import make_identity

F32 = mybir.dt.float32
BF16 = mybir.dt.bfloat16
I32 = mybir.dt.int32
AF = mybir.ActivationFunctionType
ALU = mybir.AluOpType

# Problem dims
B, NCONT, F, V, D = 16384, 13, 26, 100000, 16
H1, H2 = 400, 400
NCORES = 8
BC = B // NCORES          # 2048 rows per core
SUB = 128                 # batch subtile (partition dim)
NSUB = 4                  # subtiles per block
BLK = SUB * NSUB          # 512 rows per block
NBLK = BC // BLK          # 4 blocks per core
W17 = D + 1               # combined embedding row: 16 emb + 1 emb_first
GW = F * W17              # 442 gathered floats per row
XW = NCONT + GW           # 455 = X' row width


def _chunks(total, step=128):
    return [(s, min(step, total - s)) for s in range(0, total, step)]


def build_kernel(ncont=NCONT, f=F, v=V, d=D, h1=H1, h2=H2, bc=BC):
    """Build the per-core Bass module. Parameterized for small-config tests."""
    w17 = d + 1
    gw = f * w17
    xw = ncont + gw
    nblk = bc // BLK
    KCH = _chunks(xw)          # X' K-chunks
    MCH1 = _chunks(h1)         # L1 M-tiles == L2 K-chunks
    MCH2 = _chunks(h2)         # L2 M-tiles == out-layer K-chunks
    n_wo_ch = len(MCH2)

    nc = bacc.Bacc("TRN2", target_bir_lowering=False, debug=False,
                   dynamic_dma_scratch_size=65536)

    t_table = nc.dram_tensor("table", [f * v, w17], F32, kind="ExternalInput")
    t_idx = nc.dram_tensor("idx", [SUB, nblk * NSUB * f], I32, kind="ExternalInput")
    t_cont = nc.dram_tensor("cont", [bc, ncont], F32, kind="ExternalInput")
    t_w1 = nc.dram_tensor("w1p", [xw, h1], F32, kind="ExternalInput")
    t_w2 = nc.dram_tensor("w2", [h1, h2], F32, kind="ExternalInput")
    t_b1 = nc.dram_tensor("b1", [h1, 1], F32, kind="ExternalInput")
    t_b2 = nc.dram_tensor("b2", [h2, 1], F32, kind="ExternalInput")
    t_wo = nc.dram_tensor("wo", [128, n_wo_ch], F32, kind="ExternalInput")
    t_wc = nc.dram_tensor("wc", [128, ncont], F32, kind="ExternalInput")
    t_fs = nc.dram_tensor("fs", [128, 1], F32, kind="ExternalInput")
    t_ob = nc.dram_tensor("ob", [1, 1], F32, kind="ExternalInput")
    t_y = nc.dram_tensor("y", [nblk, 1, BLK], F32, kind="ExternalOutput")

    with tile.TileContext(nc) as tc:
        with (
            tc.tile_pool(name="wpool", bufs=1) as wpool,
            tc.tile_pool(name="xpool", bufs=2) as xpool,
            tc.tile_pool(name="hpool", bufs=2) as hpool,
            tc.tile_pool(name="fpool", bufs=2) as fpool,
            tc.tile_pool(name="opool", bufs=2) as opool,
            tc.tile_pool(name="pt_ps", bufs=2, space="PSUM") as pt_ps,
            tc.tile_pool(name="mm_ps", bufs=2, space="PSUM") as mm_ps,
            tc.tile_pool(name="o_ps", bufs=1, space="PSUM") as o_ps,
        ):
            # ---- indices first: the gather stream (the critical path)
            # must not queue behind the 1.4 MB of weight DMAs. Split per
            # block so block 0's gathers start after ~53 KB, not 212 KB. ----
            idx_all = wpool.tile([SUB, nblk * NSUB * f], I32)
            bw = NSUB * f
            for blk in range(nblk):
                nc.sync.dma_start(
                    out=idx_all[:, blk * bw : (blk + 1) * bw],
                    in_=t_idx[:, blk * bw : (blk + 1) * bw],
                )

            # ---- constants / weights (loaded once) ----
            ident = wpool.tile([128, 128], F32)
            make_identity(nc, ident)

            w1_sb = []
            for ci, (k0, ks) in enumerate(KCH):
                w1c = wpool.tile([128, h1], F32, name=f"w1c{ci}")
                nc.sync.dma_start(out=w1c[0:ks, :], in_=t_w1[k0 : k0 + ks, :])
                w1_sb.append(w1c)
            w2_sb = []
            for ci, (k0, ks) in enumerate(MCH1):
                w2c = wpool.tile([128, h2], F32, name=f"w2c{ci}")
                nc.sync.dma_start(out=w2c[0:ks, :], in_=t_w2[k0 : k0 + ks, :])
                w2_sb.append(w2c)
            b1_sb = []
            for mi, (m0, ms) in enumerate(MCH1):
                b1m = wpool.tile([128, 1], F32, name=f"b1m{mi}")
                nc.sync.dma_start(out=b1m[0:ms, :], in_=t_b1[m0 : m0 + ms, :])
                b1_sb.append(b1m)
            b2_sb = []
            for mi, (m0, ms) in enumerate(MCH2):
                b2m = wpool.tile([128, 1], F32, name=f"b2m{mi}")
                nc.sync.dma_start(out=b2m[0:ms, :], in_=t_b2[m0 : m0 + ms, :])
                b2_sb.append(b2m)
            wo_sb = wpool.tile([128, n_wo_ch], F32)
            nc.sync.dma_start(out=wo_sb[:], in_=t_wo[:])
            # one-time bf16 casts of the matmul weights (hidden under the
            # block-0 gather stream); PSUM accumulation stays fp32
            w1b_sb = []
            for ci, (k0, ks) in enumerate(KCH):
                w1b = wpool.tile([128, h1], BF16, name=f"w1b{ci}")
                nc.scalar.copy(out=w1b[0:ks, :], in_=w1_sb[ci][0:ks, :])
                w1b_sb.append(w1b)
            w2b_sb = []
            for ci, (k0, ks) in enumerate(MCH1):
                w2b = wpool.tile([128, h2], BF16, name=f"w2b{ci}")
                nc.scalar.copy(out=w2b[0:ks, :], in_=w2_sb[ci][0:ks, :])
                w2b_sb.append(w2b)
            wob_sb = wpool.tile([128, n_wo_ch], BF16)
            nc.scalar.copy(out=wob_sb[:], in_=wo_sb[:])
            # W1 rows [370:455) as one tile for the split-tail K-chunking
            # (covers features FSPL..f-1 of the last subtile)
            FSPL = 21
            csplit = ncont + FSPL * w17          # 370
            w1x = wpool.tile([128, h1], F32)
            nc.sync.dma_start(out=w1x[0 : xw - csplit, :],
                              in_=t_w1[csplit:xw, :])
            w1xb = wpool.tile([128, h1], BF16)
            nc.scalar.copy(out=w1xb[0 : xw - csplit, :],
                           in_=w1x[0 : xw - csplit, :])
            wc_sb = wpool.tile([128, ncont], F32)
            nc.sync.dma_start(out=wc_sb[:], in_=t_wc[:])
            fs_sb = wpool.tile([128, 1], F32)
            nc.sync.dma_start(out=fs_sb[:], in_=t_fs[:])
            ob_sb = wpool.tile([1, 1], F32)
            nc.sync.dma_start(out=ob_sb[:], in_=t_ob[:])

            def emit_compute(blk, xb, xb3, s0, ns):
                """DNN + FM for subtiles [s0, s0+ns) of block blk."""
                ncols = ns * SUB
                c0 = s0 * SUB
                # ---- transpose X' -> xT chunks [128, ncols] ----
                xt_sb = []
                for ci, (k0, ks) in enumerate(KCH):
                    pt = pt_ps.tile([128, BLK], F32, tag="pt")
                    for si in range(ns):
                        s = s0 + si
                        nc.tensor.transpose(
                            out=pt[0:ks, si * SUB : (si + 1) * SUB],
                            in_=xb[:, s * xw + k0 : s * xw + k0 + ks],
                            identity=ident[:],
                        )
                    xt = xpool.tile([128, BLK], BF16, tag=f"xt{ci}")
                    nc.scalar.copy(out=xt[0:ks, 0:ncols], in_=pt[0:ks, 0:ncols])
                    xt_sb.append(xt)

                # ---- FM terms (batch-major, vector engine) ----
                fmv_sb = []
                for s in range(s0, s0 + ns):
                    g3 = xb3[:, s, ncont:xw].rearrange("p (f w) -> p f w", w=w17)
                    emb_fd = g3[:, :, 0:d]                      # [128, f, d]
                    emb_df = xb3[:, s, ncont:xw].rearrange(
                        "p (f w) -> p w f", w=w17
                    )[:, 0:d, :]                                # [128, d, f]
                    first_f = g3[:, :, d : d + 1].rearrange("p f w -> p (f w)")

                    # sum over features per d, then square and reduce
                    se = fpool.tile([SUB, d], F32, tag="se")
                    nc.vector.tensor_reduce(
                        out=se[:], in_=emb_df, axis=mybir.AxisListType.X, op=ALU.add
                    )
                    se2 = fpool.tile([SUB, d], F32, tag="se2")
                    nc.vector.tensor_mul(out=se2[:], in0=se[:], in1=se[:])
                    r1 = fpool.tile([SUB, 1], F32, tag="r1")
                    nc.vector.tensor_reduce(
                        out=r1[:], in_=se2[:], axis=mybir.AxisListType.X, op=ALU.add
                    )
                    # sum of squares of all emb elements
                    sq = fpool.tile([SUB, f * d], F32, tag="sq")
                    nc.vector.tensor_mul(
                        out=sq[:].rearrange("p (f w) -> p f w", w=d),
                        in0=emb_fd, in1=emb_fd)
                    r2 = fpool.tile([SUB, 1], F32, tag="r2")
                    nc.vector.tensor_reduce(
                        out=r2[:], in_=sq[:], axis=mybir.AxisListType.X, op=ALU.add
                    )
                    # first-order embedding sum
                    rf = fpool.tile([SUB, 1], F32, tag="rf")
                    nc.vector.tensor_reduce(
                        out=rf[:], in_=first_f, axis=mybir.AxisListType.X, op=ALU.add
                    )
                    # continuous linear term
                    cw = fpool.tile([SUB, ncont], F32, tag="cw")
                    nc.vector.tensor_mul(
                        out=cw[:], in0=xb3[:, s, 0:ncont], in1=wc_sb[:])
                    r3 = fpool.tile([SUB, 1], F32, tag="r3")
                    nc.vector.tensor_reduce(
                        out=r3[:], in_=cw[:], axis=mybir.AxisListType.X, op=ALU.add
                    )
                    # fm = 0.5*(r1 - r2) + r3 + rf, scaled by w_fm
                    t1 = fpool.tile([SUB, 1], F32, tag="t1")
                    nc.vector.tensor_sub(out=t1[:], in0=r1[:], in1=r2[:])
                    t2 = fpool.tile([SUB, 1], F32, tag="t2")
                    nc.vector.tensor_scalar_mul(out=t2[:], in0=t1[:], scalar1=0.5)
                    t3 = fpool.tile([SUB, 1], F32, tag="t3")
                    nc.vector.tensor_add(out=t3[:], in0=t2[:], in1=r3[:])
                    t4 = fpool.tile([SUB, 1], F32, tag="t4")
                    nc.vector.tensor_add(out=t4[:], in0=t3[:], in1=rf[:])
                    fmv = fpool.tile([SUB, 1], F32, tag=f"fmv{s}")
                    nc.vector.tensor_mul(out=fmv[:], in0=t4[:], in1=fs_sb[:])
                    fmv_sb.append(fmv)

                # ---- L1: h1^T = relu(W1'^T X'^T + b1) ----
                h1_sb = []
                for mi, (m0, ms) in enumerate(MCH1):
                    ps1 = mm_ps.tile([128, BLK], F32, tag="ps1")
                    for ci, (k0, ks) in enumerate(KCH):
                        nc.tensor.matmul(
                            out=ps1[0:ms, 0:ncols],
                            lhsT=w1b_sb[ci][0:ks, m0 : m0 + ms],
                            rhs=xt_sb[ci][0:ks, 0:ncols],
                            start=(ci == 0), stop=(ci == len(KCH) - 1),
                        )
                    h1m = hpool.tile([128, BLK], BF16, tag=f"h1m{mi}")
                    nc.scalar.activation(
                        out=h1m[0:ms, 0:ncols], in_=ps1[0:ms, 0:ncols],
                        func=AF.Relu, bias=b1_sb[mi][0:ms, :],
                    )
                    h1_sb.append(h1m)

                # ---- L2: h2^T = relu(W2^T h1^T + b2) ----
                h2_sb = []
                for mi, (m0, ms) in enumerate(MCH2):
                    ps2 = mm_ps.tile([128, BLK], F32, tag="ps2")
                    for ci, (k0, ks) in enumerate(MCH1):
                        nc.tensor.matmul(
                            out=ps2[0:ms, 0:ncols],
                            lhsT=w2b_sb[ci][0:ks, m0 : m0 + ms],
                            rhs=h1_sb[ci][0:ks, 0:ncols],
                            start=(ci == 0), stop=(ci == len(MCH1) - 1),
                        )
                    h2m = hpool.tile([128, BLK], BF16, tag=f"h2m{mi}")
                    nc.scalar.activation(
                        out=h2m[0:ms, 0:ncols], in_=ps2[0:ms, 0:ncols],
                        func=AF.Relu, bias=b2_sb[mi][0:ms, :],
                    )
                    h2_sb.append(h2m)

                # ---- out layer: y = W_out[1:]^T h2^T + w_fm*fm + b ----
                pso = o_ps.tile([1, BLK], F32, tag="pso")
                for ci, (k0, ks) in enumerate(MCH2):
                    nc.tensor.matmul(
                        out=pso[0:1, 0:ncols],
                        lhsT=wob_sb[0:ks, ci : ci + 1],
                        rhs=h2_sb[ci][0:ks, 0:ncols],
                        start=(ci == 0), stop=(ci == len(MCH2) - 1),
                    )
                pft = o_ps.tile([1, BLK], F32, tag="pft")
                for si in range(ns):
                    nc.tensor.transpose(
                        out=pft[0:1, si * SUB : (si + 1) * SUB],
                        in_=fmv_sb[si][:, 0:1],
                        identity=ident[:],
                    )
                fsb = opool.tile([1, BLK], F32, tag="fsb")
                nc.scalar.copy(out=fsb[0:1, 0:ncols], in_=pft[0:1, 0:ncols])
                orow = opool.tile([1, BLK], F32, tag="orow")
                nc.scalar.activation(
                    out=orow[0:1, 0:ncols], in_=pso[0:1, 0:ncols],
                    func=AF.Identity, bias=ob_sb[0:1, :],
                )
                oout = opool.tile([1, BLK], F32, tag="oout")
                nc.vector.tensor_add(
                    out=oout[0:1, 0:ncols], in0=orow[0:1, 0:ncols],
                    in1=fsb[0:1, 0:ncols])
                nc.sync.dma_start(
                    out=t_y[blk][0:1, c0 : c0 + ncols], in_=oout[0:1, 0:ncols])

            def emit_compute_split(blk, xb, xb3, xlast):
                """Last subtile (s=3) with features [FSPL:f) in xlast.
                Custom K-chunks aligned to the split so chunks 0-2 depend
                only on the earlier gathers; only chunk 3 waits for the
                final five."""
                s = NSUB - 1
                ncols, c0 = SUB, s * SUB
                KC = [(0, 128, None), (128, 128, None), (256, csplit - 256, None),
                      (csplit, xw - csplit, "x")]
                xt_sb = []
                for ci, (k0, ks, src) in enumerate(KC):
                    pt = pt_ps.tile([128, BLK], F32, tag="pt")
                    if src is None:
                        nc.tensor.transpose(
                            out=pt[0:ks, 0:SUB],
                            in_=xb[:, s * xw + k0 : s * xw + k0 + ks],
                            identity=ident[:],
                        )
                    else:
                        nc.tensor.transpose(
                            out=pt[0:ks, 0:SUB], in_=xlast[:, 0:ks],
                            identity=ident[:],
                        )
                    xt = xpool.tile([128, BLK], BF16, tag=f"xt{ci}")
                    nc.scalar.copy(out=xt[0:ks, 0:ncols], in_=pt[0:ks, 0:ncols])
                    xt_sb.append(xt)

                # ---- FM, split into xb-part (features 0..FSPL) + xlast ----
                xga = xb3[:, s, ncont:csplit]
                xgb = xlast[:]
                g3a = xga.rearrange("p (f w) -> p f w", w=w17)
                g3b = xgb.rearrange("p (f w) -> p f w", w=w17)
                edf_a = xga.rearrange("p (f w) -> p w f", w=w17)[:, 0:d, :]
                edf_b = xgb.rearrange("p (f w) -> p w f", w=w17)[:, 0:d, :]
                sea = fpool.tile([SUB, d], F32, tag="sea")
                nc.vector.tensor_reduce(
                    out=sea[:], in_=edf_a, axis=mybir.AxisListType.X, op=ALU.add)
                seb = fpool.tile([SUB, d], F32, tag="seb")
                nc.vector.tensor_reduce(
                    out=seb[:], in_=edf_b, axis=mybir.AxisListType.X, op=ALU.add)
                se = fpool.tile([SUB, d], F32, tag="se3")
                nc.vector.tensor_add(out=se[:], in0=sea[:], in1=seb[:])
                se2 = fpool.tile([SUB, d], F32, tag="se23")
                nc.vector.tensor_mul(out=se2[:], in0=se[:], in1=se[:])
                r1 = fpool.tile([SUB, 1], F32, tag="r13")
                nc.vector.tensor_reduce(
                    out=r1[:], in_=se2[:], axis=mybir.AxisListType.X, op=ALU.add)
                sqa = fpool.tile([SUB, FSPL * d], F32, tag="sqa")
                nc.vector.tensor_mul(
                    out=sqa[:].rearrange("p (f w) -> p f w", w=d),
                    in0=g3a[:, :, 0:d], in1=g3a[:, :, 0:d])
                r2a = fpool.tile([SUB, 1], F32, tag="r2a")
                nc.vector.tensor_reduce(
                    out=r2a[:], in_=sqa[:], axis=mybir.AxisListType.X, op=ALU.add)
                sqb = fpool.tile([SUB, (f - FSPL) * d], F32, tag="sqb")
                nc.vector.tensor_mul(
                    out=sqb[:].rearrange("p (f w) -> p f w", w=d),
                    in0=g3b[:, :, 0:d], in1=g3b[:, :, 0:d])
                r2b = fpool.tile([SUB, 1], F32, tag="r2b")
                nc.vector.tensor_reduce(
                    out=r2b[:], in_=sqb[:], axis=mybir.AxisListType.X, op=ALU.add)
                r2 = fpool.tile([SUB, 1], F32, tag="r23")
                nc.vector.tensor_add(out=r2[:], in0=r2a[:], in1=r2b[:])
                fa = g3a[:, :, d : d + 1].rearrange("p f w -> p (f w)")
                fb = g3b[:, :, d : d + 1].rearrange("p f w -> p (f w)")
                rfa = fpool.tile([SUB, 1], F32, tag="rfa")
                nc.vector.tensor_reduce(
                    out=rfa[:], in_=fa, axis=mybir.AxisListType.X, op=ALU.add)
                rfb = fpool.tile([SUB, 1], F32, tag="rfb")
                nc.vector.tensor_reduce(
                    out=rfb[:], in_=fb, axis=mybir.AxisListType.X, op=ALU.add)
                rf = fpool.tile([SUB, 1], F32, tag="rf3")
                nc.vector.tensor_add(out=rf[:], in0=rfa[:], in1=rfb[:])
                cw = fpool.tile([SUB, ncont], F32, tag="cw3")
                nc.vector.tensor_mul(
                    out=cw[:], in0=xb3[:, s, 0:ncont], in1=wc_sb[:])
                r3 = fpool.tile([SUB, 1], F32, tag="r33")
                nc.vector.tensor_reduce(
                    out=r3[:], in_=cw[:], axis=mybir.AxisListType.X, op=ALU.add)
                t1 = fpool.tile([SUB, 1], F32, tag="t13")
                nc.vector.tensor_sub(out=t1[:], in0=r1[:], in1=r2[:])
                t2 = fpool.tile([SUB, 1], F32, tag="t23")
                nc.vector.tensor_scalar_mul(out=t2[:], in0=t1[:], scalar1=0.5)
                t3 = fpool.tile([SUB, 1], F32, tag="t33")
                nc.vector.tensor_add(out=t3[:], in0=t2[:], in1=r3[:])
                t4 = fpool.tile([SUB, 1], F32, tag="t43")
                nc.vector.tensor_add(out=t4[:], in0=t3[:], in1=rf[:])
                fmv = fpool.tile([SUB, 1], F32, tag="fmv3s")
                nc.vector.tensor_mul(out=fmv[:], in0=t4[:], in1=fs_sb[:])

                # ---- L1 with the custom chunks ----
                w1refs = [w1b_sb[0], w1b_sb[1], w1b_sb[2], w1xb]
                h1_sb = []
                for mi, (m0, ms) in enumerate(MCH1):
                    ps1 = mm_ps.tile([128, BLK], F32, tag="ps1")
                    for ci, (k0, ks, src) in enumerate(KC):
                        nc.tensor.matmul(
                            out=ps1[0:ms, 0:ncols],
                            lhsT=w1refs[ci][0:ks, m0 : m0 + ms],
                            rhs=xt_sb[ci][0:ks, 0:ncols],
                            start=(ci == 0), stop=(ci == len(KC) - 1),
                        )
                    h1m = hpool.tile([128, BLK], BF16, tag=f"h1m{mi}")
                    nc.scalar.activation(
                        out=h1m[0:ms, 0:ncols], in_=ps1[0:ms, 0:ncols],
                        func=AF.Relu, bias=b1_sb[mi][0:ms, :],
                    )
                    h1_sb.append(h1m)
                h2_sb = []
                for mi, (m0, ms) in enumerate(MCH2):
                    ps2 = mm_ps.tile([128, BLK], F32, tag="ps2")
                    for ci, (k0, ks) in enumerate(MCH1):
                        nc.tensor.matmul(
                            out=ps2[0:ms, 0:ncols],
                            lhsT=w2b_sb[ci][0:ks, m0 : m0 + ms],
                            rhs=h1_sb[ci][0:ks, 0:ncols],
                            start=(ci == 0), stop=(ci == len(MCH1) - 1),
                        )
                    h2m = hpool.tile([128, BLK], BF16, tag=f"h2m{mi}")
                    nc.scalar.activation(
                        out=h2m[0:ms, 0:ncols], in_=ps2[0:ms, 0:ncols],
                        func=AF.Relu, bias=b2_sb[mi][0:ms, :],
                    )
                    h2_sb.append(h2m)
                pso = o_ps.tile([1, BLK], F32, tag="pso")
                for ci, (k0, ks) in enumerate(MCH2):
                    nc.tensor.matmul(
                        out=pso[0:1, 0:ncols],
                        lhsT=wob_sb[0:ks, ci : ci + 1],
                        rhs=h2_sb[ci][0:ks, 0:ncols],
                        start=(ci == 0), stop=(ci == len(MCH2) - 1),
                    )
                pft = o_ps.tile([1, BLK], F32, tag="pft")
                nc.tensor.transpose(
                    out=pft[0:1, 0:SUB], in_=fmv[:, 0:1], identity=ident[:])
                fsb = opool.tile([1, BLK], F32, tag="fsb")
                nc.scalar.copy(out=fsb[0:1, 0:ncols], in_=pft[0:1, 0:ncols])
                orow = opool.tile([1, BLK], F32, tag="orow")
                nc.scalar.activation(
                    out=orow[0:1, 0:ncols], in_=pso[0:1, 0:ncols],
                    func=AF.Identity, bias=ob_sb[0:1, :],
                )
                oout = opool.tile([1, BLK], F32, tag="oout")
                nc.vector.tensor_add(
                    out=oout[0:1, 0:ncols], in0=orow[0:1, 0:ncols],
                    in1=fsb[0:1, 0:ncols])
                nc.sync.dma_start(
                    out=t_y[blk][0:1, c0 : c0 + ncols], in_=oout[0:1, 0:ncols])

            for blk in range(nblk):
                xb = xpool.tile([SUB, NSUB * xw], F32, tag="xb", bufs=4)
                xb3 = xb[:].rearrange("p (s w) -> p s w", w=xw)
                # continuous -> cols [0:ncont) of each sub-block
                cont_src = t_cont[blk * BLK : (blk + 1) * BLK, :].rearrange(
                    "(s p) c -> p s c", p=SUB
                )
                nc.sync.dma_start(out=xb3[:, :, 0:ncont], in_=cont_src)
                # gather -> cols [ncont:xw): one [128,1]-indexed indirect DMA
                # per (subtile, feature) — the only shape the SWDGE ucode
                # supports (one index per partition). The very last subtile's
                # features [FSPL:f) land in a separate tile so most of its
                # compute does not wait for the final gathers.
                last = blk == nblk - 1
                xlast = None
                if last:
                    xlast = xpool.tile([SUB, (f - FSPL) * w17], F32,
                                       tag="xlast", bufs=1, name="xlast")
                for s in range(NSUB):
                    for ff in range(f):
                        col = (blk * NSUB + s) * f + ff
                        if last and s == NSUB - 1 and ff >= FSPL:
                            out_ap = xlast[
                                :, (ff - FSPL) * w17 : (ff - FSPL + 1) * w17]
                        else:
                            c0 = s * xw + ncont + w17 * ff
                            out_ap = xb[:, c0 : c0 + w17]
                        nc.gpsimd.indirect_dma_start(
                            out=out_ap,
                            out_offset=None,
                            in_=t_table[:],
                            in_offset=IndirectOffsetOnAxis(
                                ap=idx_all[:, col : col + 1], axis=0
                            ),
                        )
                # Last block: finer compute chunks so the tail after the
                # final gather is one subtile's chain, not a whole block's.
                if not last:
                    emit_compute(blk, xb, xb3, 0, NSUB)
                else:
                    emit_compute(blk, xb, xb3, 0, 2)
                    emit_compute(blk, xb, xb3, 2, 1)
                    emit_compute_split(blk, xb, xb3, xlast)

    nc.compile()
    return nc


def prep_inputs(continuous, cat_idx, W_cont, b_cont, emb_first, emb, W1, b1,
                W2, b2, W_out, b_out, ncont=NCONT, f=F, v=V, d=D,
                h1=H1, h2=H2, ncores=NCORES):
    """Host-side: combined table, flat indices, padded weights, per-core shards."""
    b = cat_idx.shape[0]
    bc = b // ncores
    nblk = bc // BLK
    w17 = d + 1
    xw = ncont + f * w17

    table = np.concatenate(
        [np.ascontiguousarray(emb, np.float32).reshape(f * v, d),
         np.ascontiguousarray(emb_first, np.float32).reshape(f * v, 1)],
        axis=1,
    )  # [f*v, d+1]

    idx_flat = (np.asarray(cat_idx).astype(np.int64)
                + (np.arange(f, dtype=np.int64) * v)[None, :]).astype(np.int32)

    # W1 rows permuted to the gathered X' layout (zero rows at emb_first slots)
    W1 = np.asarray(W1, np.float32)
    w1p = np.zeros((xw, h1), np.float32)
    w1p[0:ncont] = W1[0:ncont]
    for ff in range(f):
        w1p[ncont + w17 * ff : ncont + w17 * ff + d] = (
            W1[ncont + d * ff : ncont + d * ff + d])

    W_out = np.asarray(W_out, np.float32)
    n_wo_ch = (h2 + 127) // 128
    wo_t = np.zeros((n_wo_ch, 128), np.float32)
    wo_t.reshape(-1)[:h2] = W_out[1:, 0]
    wo = np.ascontiguousarray(wo_t.T)

    w_fm = np.float32(W_out[0, 0])
    ob = np.float32(b_out[0] + w_fm * b_cont[0])

    common = {
        "table": table,
        "w1p": w1p,
        "w2": np.ascontiguousarray(W2, np.float32),
        "b1": np.asarray(b1, np.float32).reshape(h1, 1),
        "b2": np.asarray(b2, np.float32).reshape(h2, 1),
        "wo": wo,
        "wc": np.tile(np.asarray(W_cont, np.float32).reshape(1, ncont), (128, 1)),
        "fs": np.full((128, 1), w_fm, np.float32),
        "ob": np.array([[ob]], np.float32),
    }

    in_maps = []
    for c in range(ncores):
        rows = slice(c * bc, (c + 1) * bc)
        idx_c = idx_flat[rows].reshape(nblk * NSUB, SUB, f)  # [(blk s), p, f]
        idx_c = np.ascontiguousarray(
            idx_c.transpose(1, 0, 2).reshape(SUB, nblk * NSUB * f))
        in_maps.append({
            **common,
            "idx": idx_c,
            "cont": np.ascontiguousarray(continuous[rows], np.float32),
        })
    return in_maps


_NC_CACHE = {}


def kernel(**inputs) -> np.ndarray:
    if "nc" not in _NC_CACHE:
        _NC_CACHE["nc"] = build_kernel()
    nc = _NC_CACHE["nc"]
    in_maps = prep_inputs(**inputs)
    res = run_bass_kernel_spmd(nc, in_maps, core_ids=list(range(NCORES)))
    out = np.concatenate(
        [r["y"].reshape(BC, 1) for r in res.results], axis=0)
    return out.astype(np.float32)



# revision 34
# speedup vs baseline: 1.0063x; 1.0063x over previous
"""DeepFM (embedding_lookup) Trainium2 Bass kernel.

Sharding: data-parallel on batch across 8 NeuronCores; the combined
embedding table (~177 MB) is replicated per core.

Self-contained: hardcodes all shapes from the problem spec.
"""

import numpy as np

import concourse.bass as bass
import concourse.bacc as bacc
import concourse.mybir as mybir
import concourse.tile as tile
from concourse.bass import IndirectOffsetOnAxis
from concourse.bass_utils import run_bass_kernel_spmd
from concourse.masks import make_identity

F32 = mybir.dt.float32
BF16 = mybir.dt.bfloat16
I32 = mybir.dt.int32
AF = mybir.ActivationFunctionType
ALU = mybir.AluOpType

# Problem dims
B, NCONT, F, V, D = 16384, 13, 26, 100000, 16
H1, H2 = 400, 400
NCORES = 8
BC = B // NCORES          # 2048 rows per core
SUB = 128                 # batch subtile (partition dim)
NSUB = 4                  # subtiles per block
BLK = SUB * NSUB          # 512 rows per block
NBLK = BC // BLK          # 4 blocks per core
W17 = D + 1               # combined embedding row: 16 emb + 1 emb_first
GW = F * W17              # 442 gathered floats per row
XW = NCONT + GW           # 455 = X' row width


def _chunks(total, step=128):
    return [(s, min(step, total - s)) for s in range(0, total, step)]


def build_kernel(ncont=NCONT, f=F, v=V, d=D, h1=H1, h2=H2, bc=BC):
    """Build the per-core Bass module. Parameterized for small-config tests."""
    w17 = d + 1
    gw = f * w17
    xw = ncont + gw
    nblk = bc // BLK
    KCH = _chunks(xw)          # X' K-chunks
    MCH1 = _chunks(h1)         # L1 M-tiles == L2 K-chunks
    MCH2 = _chunks(h2)         # L2 M-tiles == out-layer K-chunks
    n_wo_ch = len(MCH2)

    nc = bacc.Bacc("TRN2", target_bir_lowering=False, debug=False,
                   dynamic_dma_scratch_size=65536)

    t_table = nc.dram_tensor("table", [f * v, w17], F32, kind="ExternalInput")
    t_idx = nc.dram_tensor("idx", [SUB, nblk * NSUB * f], I32, kind="ExternalInput")
    t_cont = nc.dram_tensor("cont", [bc, ncont], F32, kind="ExternalInput")
    t_w1 = nc.dram_tensor("w1p", [xw, h1], F32, kind="ExternalInput")
    t_w2 = nc.dram_tensor("w2", [h1, h2], F32, kind="ExternalInput")
    t_b1 = nc.dram_tensor("b1", [h1, 1], F32, kind="ExternalInput")
    t_b2 = nc.dram_tensor("b2", [h2, 1], F32, kind="ExternalInput")
    t_wo = nc.dram_tensor("wo", [128, n_wo_ch], F32, kind="ExternalInput")
    t_wc = nc.dram_tensor("wc", [128, ncont], F32, kind="ExternalInput")
    t_fs = nc.dram_tensor("fs", [128, 1], F32, kind="ExternalInput")
    t_ob = nc.dram_tensor("ob", [1, 1], F32, kind="ExternalInput")
    t_y = nc.dram_tensor("y", [nblk, 1, BLK], F32, kind="ExternalOutput")

    with tile.TileContext(nc) as tc:
        with (
            tc.tile_pool(name="wpool", bufs=1) as wpool,
            tc.tile_pool(name="xpool", bufs=2) as xpool,
            tc.tile_pool(name="hpool", bufs=2) as hpool,
            tc.tile_pool(name="fpool", bufs=2) as fpool,
            tc.tile_pool(name="opool", bufs=2) as opool,
            tc.tile_pool(name="pt_ps", bufs=2, space="PSUM") as pt_ps,
            tc.tile_pool(name="mm_ps", bufs=2, space="PSUM") as mm_ps,
            tc.tile_pool(name="o_ps", bufs=1, space="PSUM") as o_ps,
        ):
            # ---- indices first: the gather stream (the critical path)
            # must not queue behind the 1.4 MB of weight DMAs. Split per
            # block so block 0's gathers start after ~53 KB, not 212 KB. ----
            idx_all = wpool.tile([SUB, nblk * NSUB * f], I32)
            bw = NSUB * f
            for blk in range(nblk):
                nc.sync.dma_start(
                    out=idx_all[:, blk * bw : (blk + 1) * bw],
                    in_=t_idx[:, blk * bw : (blk + 1) * bw],
                )

            # ---- constants / weights (loaded once). The identity matrix is
            # WRITTEN later (after block 0's gathers) — make_identity's
            # memset/iota/affine_select run on the gpsimd queue and would
            # otherwise delay the first gather; it isn't consumed until the
            # first transpose ~150µs in. ----
            ident = wpool.tile([128, 128], F32)

            w1_sb = []
            for ci, (k0, ks) in enumerate(KCH):
                w1c = wpool.tile([128, h1], F32, name=f"w1c{ci}")
                nc.sync.dma_start(out=w1c[0:ks, :], in_=t_w1[k0 : k0 + ks, :])
                w1_sb.append(w1c)
            w2_sb = []
            for ci, (k0, ks) in enumerate(MCH1):
                w2c = wpool.tile([128, h2], F32, name=f"w2c{ci}")
                nc.sync.dma_start(out=w2c[0:ks, :], in_=t_w2[k0 : k0 + ks, :])
                w2_sb.append(w2c)
            b1_sb = []
            for mi, (m0, ms) in enumerate(MCH1):
                b1m = wpool.tile([128, 1], F32, name=f"b1m{mi}")
                nc.sync.dma_start(out=b1m[0:ms, :], in_=t_b1[m0 : m0 + ms, :])
                b1_sb.append(b1m)
            b2_sb = []
            for mi, (m0, ms) in enumerate(MCH2):
                b2m = wpool.tile([128, 1], F32, name=f"b2m{mi}")
                nc.sync.dma_start(out=b2m[0:ms, :], in_=t_b2[m0 : m0 + ms, :])
                b2_sb.append(b2m)
            wo_sb = wpool.tile([128, n_wo_ch], F32)
            nc.sync.dma_start(out=wo_sb[:], in_=t_wo[:])
            # one-time bf16 casts of the matmul weights (hidden under the
            # block-0 gather stream); PSUM accumulation stays fp32
            w1b_sb = []
            for ci, (k0, ks) in enumerate(KCH):
                w1b = wpool.tile([128, h1], BF16, name=f"w1b{ci}")
                nc.scalar.copy(out=w1b[0:ks, :], in_=w1_sb[ci][0:ks, :])
                w1b_sb.append(w1b)
            w2b_sb = []
            for ci, (k0, ks) in enumerate(MCH1):
                w2b = wpool.tile([128, h2], BF16, name=f"w2b{ci}")
                nc.scalar.copy(out=w2b[0:ks, :], in_=w2_sb[ci][0:ks, :])
                w2b_sb.append(w2b)
            wob_sb = wpool.tile([128, n_wo_ch], BF16)
            nc.scalar.copy(out=wob_sb[:], in_=wo_sb[:])
            # W1 rows [370:455) as one tile for the split-tail K-chunking
            # (covers features FSPL..f-1 of the last subtile)
            FSPL = 21
            csplit = ncont + FSPL * w17          # 370
            w1x = wpool.tile([128, h1], F32)
            nc.sync.dma_start(out=w1x[0 : xw - csplit, :],
                              in_=t_w1[csplit:xw, :])
            w1xb = wpool.tile([128, h1], BF16)
            nc.scalar.copy(out=w1xb[0 : xw - csplit, :],
                           in_=w1x[0 : xw - csplit, :])
            wc_sb = wpool.tile([128, ncont], F32)
            nc.sync.dma_start(out=wc_sb[:], in_=t_wc[:])
            fs_sb = wpool.tile([128, 1], F32)
            nc.sync.dma_start(out=fs_sb[:], in_=t_fs[:])
            ob_sb = wpool.tile([1, 1], F32)
            nc.sync.dma_start(out=ob_sb[:], in_=t_ob[:])

            def emit_compute(blk, xb, xb3, s0, ns):
                """DNN + FM for subtiles [s0, s0+ns) of block blk."""
                ncols = ns * SUB
                c0 = s0 * SUB
                # ---- transpose X' -> xT chunks [128, ncols] ----
                xt_sb = []
                for ci, (k0, ks) in enumerate(KCH):
                    pt = pt_ps.tile([128, BLK], F32, tag="pt")
                    for si in range(ns):
                        s = s0 + si
                        nc.tensor.transpose(
                            out=pt[0:ks, si * SUB : (si + 1) * SUB],
                            in_=xb[:, s * xw + k0 : s * xw + k0 + ks],
                            identity=ident[:],
                        )
                    xt = xpool.tile([128, BLK], BF16, tag=f"xt{ci}")
                    nc.scalar.copy(out=xt[0:ks, 0:ncols], in_=pt[0:ks, 0:ncols])
                    xt_sb.append(xt)

                # ---- FM terms (batch-major, vector engine) ----
                fmv_sb = []
                for s in range(s0, s0 + ns):
                    g3 = xb3[:, s, ncont:xw].rearrange("p (f w) -> p f w", w=w17)
                    emb_fd = g3[:, :, 0:d]                      # [128, f, d]
                    emb_df = xb3[:, s, ncont:xw].rearrange(
                        "p (f w) -> p w f", w=w17
                    )[:, 0:d, :]                                # [128, d, f]
                    first_f = g3[:, :, d : d + 1].rearrange("p f w -> p (f w)")

                    # sum over features per d, then square and reduce
                    se = fpool.tile([SUB, d], F32, tag="se")
                    nc.vector.tensor_reduce(
                        out=se[:], in_=emb_df, axis=mybir.AxisListType.X, op=ALU.add
                    )
                    se2 = fpool.tile([SUB, d], F32, tag="se2")
                    nc.vector.tensor_mul(out=se2[:], in0=se[:], in1=se[:])
                    r1 = fpool.tile([SUB, 1], F32, tag="r1")
                    nc.vector.tensor_reduce(
                        out=r1[:], in_=se2[:], axis=mybir.AxisListType.X, op=ALU.add
                    )
                    # sum of squares of all emb elements
                    sq = fpool.tile([SUB, f * d], F32, tag="sq")
                    nc.vector.tensor_mul(
                        out=sq[:].rearrange("p (f w) -> p f w", w=d),
                        in0=emb_fd, in1=emb_fd)
                    r2 = fpool.tile([SUB, 1], F32, tag="r2")
                    nc.vector.tensor_reduce(
                        out=r2[:], in_=sq[:], axis=mybir.AxisListType.X, op=ALU.add
                    )
                    # first-order embedding sum
                    rf = fpool.tile([SUB, 1], F32, tag="rf")
                    nc.vector.tensor_reduce(
                        out=rf[:], in_=first_f, axis=mybir.AxisListType.X, op=ALU.add
                    )
                    # continuous linear term
                    cw = fpool.tile([SUB, ncont], F32, tag="cw")
                    nc.vector.tensor_mul(
                        out=cw[:], in0=xb3[:, s, 0:ncont], in1=wc_sb[:])
                    r3 = fpool.tile([SUB, 1], F32, tag="r3")
                    nc.vector.tensor_reduce(
                        out=r3[:], in_=cw[:], axis=mybir.AxisListType.X, op=ALU.add
                    )
                    # fm = 0.5*(r1 - r2) + r3 + rf, scaled by w_fm
                    t1 = fpool.tile([SUB, 1], F32, tag="t1")
                    nc.vector.tensor_sub(out=t1[:], in0=r1[:], in1=r2[:])
                    t2 = fpool.tile([SUB, 1], F32, tag="t2")
                    nc.vector.tensor_scalar_mul(out=t2[:], in0=t1[:], scalar1=0.5)
                    t3 = fpool.tile([SUB, 1], F32, tag="t3")
                    nc.vector.tensor_add(out=t3[:], in0=t2[:], in1=r3[:])
                    t4 = fpool.tile([SUB, 1], F32, tag="t4")
                    nc.vector.tensor_add(out=t4[:], in0=t3[:], in1=rf[:])
                    fmv = fpool.tile([SUB, 1], F32, tag=f"fmv{s}")
                    nc.vector.tensor_mul(out=fmv[:], in0=t4[:], in1=fs_sb[:])
                    fmv_sb.append(fmv)

                # ---- L1: h1^T = relu(W1'^T X'^T + b1) ----
                h1_sb = []
                for mi, (m0, ms) in enumerate(MCH1):
                    ps1 = mm_ps.tile([128, BLK], F32, tag="ps1")
                    for ci, (k0, ks) in enumerate(KCH):
                        nc.tensor.matmul(
                            out=ps1[0:ms, 0:ncols],
                            lhsT=w1b_sb[ci][0:ks, m0 : m0 + ms],
                            rhs=xt_sb[ci][0:ks, 0:ncols],
                            start=(ci == 0), stop=(ci == len(KCH) - 1),
                        )
                    h1m = hpool.tile([128, BLK], BF16, tag=f"h1m{mi}")
                    nc.scalar.activation(
                        out=h1m[0:ms, 0:ncols], in_=ps1[0:ms, 0:ncols],
                        func=AF.Relu, bias=b1_sb[mi][0:ms, :],
                    )
                    h1_sb.append(h1m)

                # ---- L2: h2^T = relu(W2^T h1^T + b2) ----
                h2_sb = []
                for mi, (m0, ms) in enumerate(MCH2):
                    ps2 = mm_ps.tile([128, BLK], F32, tag="ps2")
                    for ci, (k0, ks) in enumerate(MCH1):
                        nc.tensor.matmul(
                            out=ps2[0:ms, 0:ncols],
                            lhsT=w2b_sb[ci][0:ks, m0 : m0 + ms],
                            rhs=h1_sb[ci][0:ks, 0:ncols],
                            start=(ci == 0), stop=(ci == len(MCH1) - 1),
                        )
                    h2m = hpool.tile([128, BLK], BF16, tag=f"h2m{mi}")
                    nc.scalar.activation(
                        out=h2m[0:ms, 0:ncols], in_=ps2[0:ms, 0:ncols],
                        func=AF.Relu, bias=b2_sb[mi][0:ms, :],
                    )
                    h2_sb.append(h2m)

                # ---- out layer: y = W_out[1:]^T h2^T + w_fm*fm + b ----
                pso = o_ps.tile([1, BLK], F32, tag="pso")
                for ci, (k0, ks) in enumerate(MCH2):
                    nc.tensor.matmul(
                        out=pso[0:1, 0:ncols],
                        lhsT=wob_sb[0:ks, ci : ci + 1],
                        rhs=h2_sb[ci][0:ks, 0:ncols],
                        start=(ci == 0), stop=(ci == len(MCH2) - 1),
                    )
                pft = o_ps.tile([1, BLK], F32, tag="pft")
                for si in range(ns):
                    nc.tensor.transpose(
                        out=pft[0:1, si * SUB : (si + 1) * SUB],
                        in_=fmv_sb[si][:, 0:1],
                        identity=ident[:],
                    )
                fsb = opool.tile([1, BLK], F32, tag="fsb")
                nc.scalar.copy(out=fsb[0:1, 0:ncols], in_=pft[0:1, 0:ncols])
                orow = opool.tile([1, BLK], F32, tag="orow")
                nc.scalar.activation(
                    out=orow[0:1, 0:ncols], in_=pso[0:1, 0:ncols],
                    func=AF.Identity, bias=ob_sb[0:1, :],
                )
                oout = opool.tile([1, BLK], F32, tag="oout")
                nc.vector.tensor_add(
                    out=oout[0:1, 0:ncols], in0=orow[0:1, 0:ncols],
                    in1=fsb[0:1, 0:ncols])
                nc.sync.dma_start(
                    out=t_y[blk][0:1, c0 : c0 + ncols], in_=oout[0:1, 0:ncols])

            def emit_compute_split(blk, xb, xb3, xlast):
                """Last subtile (s=3) with features [FSPL:f) in xlast.
                Custom K-chunks aligned to the split so chunks 0-2 depend
                only on the earlier gathers; only chunk 3 waits for the
                final five."""
                s = NSUB - 1
                ncols, c0 = SUB, s * SUB
                KC = [(0, 128, None), (128, 128, None), (256, csplit - 256, None),
                      (csplit, xw - csplit, "x")]
                xt_sb = []
                for ci, (k0, ks, src) in enumerate(KC):
                    pt = pt_ps.tile([128, BLK], F32, tag="pt")
                    if src is None:
                        nc.tensor.transpose(
                            out=pt[0:ks, 0:SUB],
                            in_=xb[:, s * xw + k0 : s * xw + k0 + ks],
                            identity=ident[:],
                        )
                    else:
                        nc.tensor.transpose(
                            out=pt[0:ks, 0:SUB], in_=xlast[:, 0:ks],
                            identity=ident[:],
                        )
                    xt = xpool.tile([128, BLK], BF16, tag=f"xt{ci}")
                    nc.scalar.copy(out=xt[0:ks, 0:ncols], in_=pt[0:ks, 0:ncols])
                    xt_sb.append(xt)

                # ---- FM, split into xb-part (features 0..FSPL) + xlast ----
                xga = xb3[:, s, ncont:csplit]
                xgb = xlast[:]
                g3a = xga.rearrange("p (f w) -> p f w", w=w17)
                g3b = xgb.rearrange("p (f w) -> p f w", w=w17)
                edf_a = xga.rearrange("p (f w) -> p w f", w=w17)[:, 0:d, :]
                edf_b = xgb.rearrange("p (f w) -> p w f", w=w17)[:, 0:d, :]
                sea = fpool.tile([SUB, d], F32, tag="sea")
                nc.vector.tensor_reduce(
                    out=sea[:], in_=edf_a, axis=mybir.AxisListType.X, op=ALU.add)
                seb = fpool.tile([SUB, d], F32, tag="seb")
                nc.vector.tensor_reduce(
                    out=seb[:], in_=edf_b, axis=mybir.AxisListType.X, op=ALU.add)
                se = fpool.tile([SUB, d], F32, tag="se3")
                nc.vector.tensor_add(out=se[:], in0=sea[:], in1=seb[:])
                se2 = fpool.tile([SUB, d], F32, tag="se23")
                nc.vector.tensor_mul(out=se2[:], in0=se[:], in1=se[:])
                r1 = fpool.tile([SUB, 1], F32, tag="r13")
                nc.vector.tensor_reduce(
                    out=r1[:], in_=se2[:], axis=mybir.AxisListType.X, op=ALU.add)
                sqa = fpool.tile([SUB, FSPL * d], F32, tag="sqa")
                nc.vector.tensor_mul(
                    out=sqa[:].rearrange("p (f w) -> p f w", w=d),
                    in0=g3a[:, :, 0:d], in1=g3a[:, :, 0:d])
                r2a = fpool.tile([SUB, 1], F32, tag="r2a")
                nc.vector.tensor_reduce(
                    out=r2a[:], in_=sqa[:], axis=mybir.AxisListType.X, op=ALU.add)
                sqb = fpool.tile([SUB, (f - FSPL) * d], F32, tag="sqb")
                nc.vector.tensor_mul(
                    out=sqb[:].rearrange("p (f w) -> p f w", w=d),
                    in0=g3b[:, :, 0:d], in1=g3b[:, :, 0:d])
                r2b = fpool.tile([SUB, 1], F32, tag="r2b")
                nc.vector.tensor_reduce(
                    out=r2b[:], in_=sqb[:], axis=mybir.AxisListType.X, op=ALU.add)
                r2 = fpool.tile([SUB, 1], F32, tag="r23")
                nc.vector.tensor_add(out=r2[:], in0=r2a[:], in1=r2b[:])
                fa = g3a[:, :, d : d + 1].rearrange("p f w -> p (f w)")
                fb = g3b[:, :, d : d + 1].rearrange("p f w -> p (f w)")
                rfa = fpool.tile([SUB, 1], F32, tag="rfa")
                nc.vector.tensor_reduce(
                    out=rfa[:], in_=fa, axis=mybir.AxisListType.X, op=ALU.add)
                rfb = fpool.tile([SUB, 1], F32, tag="rfb")
                nc.vector.tensor_reduce(
                    out=rfb[:], in_=fb, axis=mybir.AxisListType.X, op=ALU.add)
                rf = fpool.tile([SUB, 1], F32, tag="rf3")
                nc.vector.tensor_add(out=rf[:], in0=rfa[:], in1=rfb[:])
                cw = fpool.tile([SUB, ncont], F32, tag="cw3")
                nc.vector.tensor_mul(
                    out=cw[:], in0=xb3[:, s, 0:ncont], in1=wc_sb[:])
                r3 = fpool.tile([SUB, 1], F32, tag="r33")
                nc.vector.tensor_reduce(
                    out=r3[:], in_=cw[:], axis=mybir.AxisListType.X, op=ALU.add)
                t1 = fpool.tile([SUB, 1], F32, tag="t13")
                nc.vector.tensor_sub(out=t1[:], in0=r1[:], in1=r2[:])
                t2 = fpool.tile([SUB, 1], F32, tag="t23")
                nc.vector.tensor_scalar_mul(out=t2[:], in0=t1[:], scalar1=0.5)
                t3 = fpool.tile([SUB, 1], F32, tag="t33")
                nc.vector.tensor_add(out=t3[:], in0=t2[:], in1=r3[:])
                t4 = fpool.tile([SUB, 1], F32, tag="t43")
                nc.vector.tensor_add(out=t4[:], in0=t3[:], in1=rf[:])
                fmv = fpool.tile([SUB, 1], F32, tag="fmv3s")
                nc.vector.tensor_mul(out=fmv[:], in0=t4[:], in1=fs_sb[:])

                # ---- L1 with the custom chunks ----
                w1refs = [w1b_sb[0], w1b_sb[1], w1b_sb[2], w1xb]
                h1_sb = []
                for mi, (m0, ms) in enumerate(MCH1):
                    ps1 = mm_ps.tile([128, BLK], F32, tag="ps1")
                    for ci, (k0, ks, src) in enumerate(KC):
                        nc.tensor.matmul(
                            out=ps1[0:ms, 0:ncols],
                            lhsT=w1refs[ci][0:ks, m0 : m0 + ms],
                            rhs=xt_sb[ci][0:ks, 0:ncols],
                            start=(ci == 0), stop=(ci == len(KC) - 1),
                        )
                    h1m = hpool.tile([128, BLK], BF16, tag=f"h1m{mi}")
                    nc.scalar.activation(
                        out=h1m[0:ms, 0:ncols], in_=ps1[0:ms, 0:ncols],
                        func=AF.Relu, bias=b1_sb[mi][0:ms, :],
                    )
                    h1_sb.append(h1m)
                h2_sb = []
                for mi, (m0, ms) in enumerate(MCH2):
                    ps2 = mm_ps.tile([128, BLK], F32, tag="ps2")
                    for ci, (k0, ks) in enumerate(MCH1):
                        nc.tensor.matmul(
                            out=ps2[0:ms, 0:ncols],
                            lhsT=w2b_sb[ci][0:ks, m0 : m0 + ms],
                            rhs=h1_sb[ci][0:ks, 0:ncols],
                            start=(ci == 0), stop=(ci == len(MCH1) - 1),
                        )
                    h2m = hpool.tile([128, BLK], BF16, tag=f"h2m{mi}")
                    nc.scalar.activation(
                        out=h2m[0:ms, 0:ncols], in_=ps2[0:ms, 0:ncols],
                        func=AF.Relu, bias=b2_sb[mi][0:ms, :],
                    )
                    h2_sb.append(h2m)
                pso = o_ps.tile([1, BLK], F32, tag="pso")
                for ci, (k0, ks) in enumerate(MCH2):
                    nc.tensor.matmul(
                        out=pso[0:1, 0:ncols],
                        lhsT=wob_sb[0:ks, ci : ci + 1],
                        rhs=h2_sb[ci][0:ks, 0:ncols],
                        start=(ci == 0), stop=(ci == len(MCH2) - 1),
                    )
                pft = o_ps.tile([1, BLK], F32, tag="pft")
                nc.tensor.transpose(
                    out=pft[0:1, 0:SUB], in_=fmv[:, 0:1], identity=ident[:])
                fsb = opool.tile([1, BLK], F32, tag="fsb")
                nc.scalar.copy(out=fsb[0:1, 0:ncols], in_=pft[0:1, 0:ncols])
                orow = opool.tile([1, BLK], F32, tag="orow")
                nc.scalar.activation(
                    out=orow[0:1, 0:ncols], in_=pso[0:1, 0:ncols],
                    func=AF.Identity, bias=ob_sb[0:1, :],
                )
                oout = opool.tile([1, BLK], F32, tag="oout")
                nc.vector.tensor_add(
                    out=oout[0:1, 0:ncols], in0=orow[0:1, 0:ncols],
                    in1=fsb[0:1, 0:ncols])
                nc.sync.dma_start(
                    out=t_y[blk][0:1, c0 : c0 + ncols], in_=oout[0:1, 0:ncols])

            for blk in range(nblk):
                xb = xpool.tile([SUB, NSUB * xw], F32, tag="xb", bufs=4)
                xb3 = xb[:].rearrange("p (s w) -> p s w", w=xw)
                # continuous -> cols [0:ncont) of each sub-block
                cont_src = t_cont[blk * BLK : (blk + 1) * BLK, :].rearrange(
                    "(s p) c -> p s c", p=SUB
                )
                nc.sync.dma_start(out=xb3[:, :, 0:ncont], in_=cont_src)
                # gather -> cols [ncont:xw): one [128,1]-indexed indirect DMA
                # per (subtile, feature) — the only shape the SWDGE ucode
                # supports (one index per partition). The very last subtile's
                # features [FSPL:f) land in a separate tile so most of its
                # compute does not wait for the final gathers.
                last = blk == nblk - 1
                xlast = None
                if last:
                    xlast = xpool.tile([SUB, (f - FSPL) * w17], F32,
                                       tag="xlast", bufs=1, name="xlast")
                for s in range(NSUB):
                    for ff in range(f):
                        col = (blk * NSUB + s) * f + ff
                        if last and s == NSUB - 1 and ff >= FSPL:
                            out_ap = xlast[
                                :, (ff - FSPL) * w17 : (ff - FSPL + 1) * w17]
                        else:
                            c0 = s * xw + ncont + w17 * ff
                            out_ap = xb[:, c0 : c0 + w17]
                        nc.gpsimd.indirect_dma_start(
                            out=out_ap,
                            out_offset=None,
                            in_=t_table[:],
                            in_offset=IndirectOffsetOnAxis(
                                ap=idx_all[:, col : col + 1], axis=0
                            ),
                        )
                # Identity written after block 0's gathers are queued so it
                # doesn't sit ahead of them on the gpsimd queue.
                if blk == 0:
                    make_identity(nc, ident)
                # Last block: finer compute chunks so the tail after the
                # final gather is one subtile's chain, not a whole block's.
                if not last:
                    emit_compute(blk, xb, xb3, 0, NSUB)
                else:
                    emit_compute(blk, xb, xb3, 0, 2)
                    emit_compute(blk, xb, xb3, 2, 1)
                    emit_compute_split(blk, xb, xb3, xlast)

    nc.compile()
    return nc


def prep_inputs(continuous, cat_idx, W_cont, b_cont, emb_first, emb, W1, b1,
                W2, b2, W_out, b_out, ncont=NCONT, f=F, v=V, d=D,
                h1=H1, h2=H2, ncores=NCORES):
    """Host-side: combined table, flat indices, padded weights, per-core shards."""
    b = cat_idx.shape[0]
    bc = b // ncores
    nblk = bc // BLK
    w17 = d + 1
    xw = ncont + f * w17

    table = np.concatenate(
        [np.ascontiguousarray(emb, np.float32).reshape(f * v, d),
         np.ascontiguousarray(emb_first, np.float32).reshape(f * v, 1)],
        axis=1,
    )  # [f*v, d+1]

    idx_flat = (np.asarray(cat_idx).astype(np.int64)
                + (np.arange(f, dtype=np.int64) * v)[None, :]).astype(np.int32)

    # W1 rows permuted to the gathered X' layout (zero rows at emb_first slots)
    W1 = np.asarray(W1, np.float32)
    w1p = np.zeros((xw, h1), np.float32)
    w1p[0:ncont] = W1[0:ncont]
    for ff in range(f):
        w1p[ncont + w17 * ff : ncont + w17 * ff + d] = (
            W1[ncont + d * ff : ncont + d * ff + d])

    W_out = np.asarray(W_out, np.float32)
    n_wo_ch = (h2 + 127) // 128
    wo_t = np.zeros((n_wo_ch, 128), np.float32)
    wo_t.reshape(-1)[:h2] = W_out[1:, 0]
    wo = np.ascontiguousarray(wo_t.T)

    w_fm = np.float32(W_out[0, 0])
    ob = np.float32(b_out[0] + w_fm * b_cont[0])

    common = {
        "table": table,
        "w1p": w1p,
        "w2": np.ascontiguousarray(W2, np.float32),
        "b1": np.asarray(b1, np.float32).reshape(h1, 1),
        "b2": np.asarray(b2, np.float32).reshape(h2, 1),
        "wo": wo,
        "wc": np.tile(np.asarray(W_cont, np.float32).reshape(1, ncont), (128, 1)),
        "fs": np.full((128, 1), w_fm, np.float32),
        "ob": np.array([[ob]], np.float32),
    }

    in_maps = []
    for c in range(ncores):
        rows = slice(c * bc, (c + 1) * bc)
        idx_c = idx_flat[rows].reshape(nblk * NSUB, SUB, f)  # [(blk s), p, f]
        idx_c = np.ascontiguousarray(
            idx_c.transpose(1, 0, 2).reshape(SUB, nblk * NSUB * f))
        in_maps.append({
            **common,
            "idx": idx_c,
            "cont": np.ascontiguousarray(continuous[rows], np.float32),
        })
    return in_maps


_NC_CACHE = {}


def kernel(**inputs) -> np.ndarray:
    if "nc" not in _NC_CACHE:
        _NC_CACHE["nc"] = build_kernel()
    nc = _NC_CACHE["nc"]
    in_maps = prep_inputs(**inputs)
    res = run_bass_kernel_spmd(nc, in_maps, core_ids=list(range(NCORES)))
    out = np.concatenate(
        [r["y"].reshape(BC, 1) for r in res.results], axis=0)
    return out.astype(np.float32)

